# revision 2
# baseline (speedup 1.0000x reference)
"""Trainium2 Bass kernel for nn_Classifier_69818988363910 (segment_reduce).

Reference computation (after dead-code elimination):
    local = relu(x @ W1^T)                        # [60000, 2048]
    feats = local.reshape(2000, 30, 2048).mean(1) # [2000, 2048]
    logits = concat(feats, feats) @ Wlin^T        # [2000, 1000]
           = feats @ (Wlin[:, :2048] + Wlin[:, 2048:])^T
y / W2 are computed but unused in the reference (original-code bug), so the
output depends only on x, W1, Wlin.

Sharding: data-parallel over the 8 NeuronCores along T (7500 rows = 250
segments per core); W1 / Wc replicated. No collectives; host gathers.

Device kernel per core (fp32 accumulation in PSUM throughout):
    MM1 on PE:   z[e, t] = sum_d W1T[d, e] * xT[d, t]
                 bf16 mode: 8 k-tiles of 128;  fp8 mode: 4 DoubleRow
                 super-k-tiles of 256 (2x PE rate)
    relu on ACT: psum -> sbuf
    pool on DVE: tensor_reduce over [128, segs, 30] view (sum; the 1/30
                 mean scale and the fp8 W1 pre-scale are folded into Wc
                 on the host)
    MM2 on PE:   logits[s, c] = sum_e featsT[e, s] * WcT[e, c]  (bf16)

Schedule notes (tuned against the ntff profile):
  * ~40 dummy N=64 matmuls on a zero tile run during the startup DMA
    window so the PE HAM clock-gate un-throttles (1.2 -> 2.4 GHz)
    before the first real matmul.
  * Chunks are 14 x 480 + 2 x 390 so no chunk is narrow enough to be
    LDWEIGHTS-bound (the old 300-wide tail chunk was).
  * MM2 is split into two rounds: segs 0..127 right after chunk 7
    (its output DMA drains mid-kernel) and segs 128..249 at the end in
    four 250-column PSUM banks whose copy+DMA pipeline across engines,
    so almost no output traffic is left after the last matmul.
  * Chunk 0 is issued k-outer across 8 parallel PSUM groups so the PE
    can start as soon as the first weight k-tile lands.
"""

import os

import numpy as np
import ml_dtypes

BF16 = ml_dtypes.bfloat16
FP8 = ml_dtypes.float8_e4m3

MODE = os.environ.get("BASS_KERNEL_MODE", "fp8")    # "bf16" | "fp8"
W1_SCALE = 32.0                                     # fp8 mode: keep W1 out of subnormals

N_CORES = 8
T, D, E, C, J = 60000, 1024, 2048, 1000, 30
T_LOC = T // N_CORES          # 7500 rows per core
S_LOC = T_LOC // J            # 250 segments per core
CHUNK = 480                   # max t-chunk (psum bank limit is 512 f32)
CHUNK_WIDTHS = [480] * 14 + [390, 390]   # 14x16 + 2x13 = 250 segments
E_TILES = E // 128            # 16
R1_SEGS = 128                 # MM2 round 1: segs 0..127 (after chunk 7)
R2_SEGS = S_LOC - R1_SEGS     # MM2 round 2: segs 128..249 (122 rows)
WARMUP_MMS = 40               # dummy matmuls to flip the HAM clock gate

_cache = {}


def _build(mode):
    from concourse import bacc, mybir
    from concourse.tile import TileContext

    f32 = mybir.dt.float32
    bf16 = mybir.dt.bfloat16
    fp8 = mybir.dt.float8e4
    in_dt = fp8 if mode == "fp8" else bf16
    KT = 4 if mode == "fp8" else 8          # accumulation steps per psum group
    perf = mybir.MatmulPerfMode.DoubleRow if mode == "fp8" else None

    nc = bacc.Bacc(trn_type="TRN2", target_bir_lowering=False, debug=False,
                   num_devices=N_CORES, num_swdge_queues=4)

    # x shard pre-chunked on the host to [chunk][p=128][d_tile][t] so each
    # chunk is one partition-contiguous DMA
    xt_d = nc.declare_dram_parameter("xt", [D * T_LOC], in_dt, isOutput=False)
    w1t_d = nc.declare_dram_parameter("w1t", [D, E], in_dt, isOutput=False)
    wct_d = nc.declare_dram_parameter("wct", [E, C], bf16, isOutput=False)
    out_d = nc.declare_dram_parameter("out", [S_LOC, C], f32, isOutput=True)

    chunks = []
    t0 = 0
    for w in CHUNK_WIDTHS:
        chunks.append((t0, w))
        t0 += w
    assert t0 == T_LOC

    if mode == "fp8":
        # weight super-k-tile kt covers d = kt*256 + j*128 + p
        w1t_v = w1t_d[:, :].rearrange("(kt j p) e -> p kt j e", j=2, p=128)
    else:
        w1t_v = w1t_d[:, :].rearrange("(kt p) e -> p kt e", p=128)

    with TileContext(nc) as tc:
        with (
            tc.tile_pool(name="xin", bufs=2) as px,
            tc.tile_pool(name="wgt", bufs=1) as pw,
            tc.tile_pool(name="zrl", bufs=4) as pz,
        ):
            # --- PE warm-up: the HAM clock gate keeps the PE at 1.2 GHz
            # until ~3.4us of sustained matmul activity.  Burn that window
            # on dummy matmuls over a zeroed tile while the first weight /
            # x DMAs are still in flight, so every real matmul runs at
            # 2.4 GHz.  (vector does the memset: gpsimd is busy issuing
            # the W1 DMAs and scalar/sync feed the x chunks.)
            warm = pw.tile([128, 128], in_dt, tag="warm", name="warm")
            nc.vector.memset(warm, 0)

            # --- weight tiles on GpSimd's queues: they start streaming at
            # preamble-end and trickle in k-ascending order while the HWDGE
            # engines (sync/scalar) feed the x chunks.  The first piece
            # (kt0, e-cols 0:256) rides Scalar's HWDGE, which spins up
            # ~0.5us before the GpSimd SWDGE queues, so the very first
            # matmul is gated only by x chunk 0.
            if mode == "fp8":
                w1_sb = [pw.tile([128, 2, E], fp8, tag=f"w1_{k}", name=f"w1_{k}")
                         for k in range(KT)]
            else:
                w1_sb = [pw.tile([128, E], bf16, tag=f"w1_{k}", name=f"w1_{k}")
                         for k in range(KT)]
            q = E // 4
            first = (0, slice(0, 256))
            order = ([(0, slice(256, q)), (0, slice(q, 2 * q))]
                     + [(k, slice(0, E // 2)) for k in range(1, KT)]
                     + [(0, slice(2 * q, 3 * q)), (0, slice(3 * q, E))]
                     + [(k, slice(E // 2, E)) for k in range(1, KT)])

            def w1_dma(eng, kt, hs):
                if mode == "fp8":
                    eng.dma_start(out=w1_sb[kt][:, :, hs], in_=w1t_v[:, kt, :, hs])
                else:
                    eng.dma_start(out=w1_sb[kt][:, hs], in_=w1t_v[:, kt, hs])

            w1_dma(nc.scalar, *first)
            for kt, hs in order:
                w1_dma(nc.gpsimd, kt, hs)

            feats = [pw.tile([128, S_LOC], f32, tag=f"fs_{e}", name=f"fs_{e}")
                     for e in range(E_TILES)]

            def lhsT(kt, e):
                if mode == "fp8":
                    return w1_sb[kt][:, :, e * 128:(e + 1) * 128]
                return w1_sb[kt][:, e * 128:(e + 1) * 128]

            def rhs(xt, kt, w):
                if mode == "fp8":
                    return xt[:, 2 * kt:2 * kt + 2, :w]
                return xt[:, kt, :w]

            featsb = [pw.tile([128, S_LOC], bf16, tag=f"fb_{e}", name=f"fb_{e}")
                      for e in range(E_TILES)]

            def relu_pool(ps, w, e, s0, conv=None):
                segs = w // J
                zr = pz.tile([128, CHUNK], f32, tag="zr", name="zr")
                nc.scalar.activation(zr[:, :w], ps[:, :w],
                                     mybir.ActivationFunctionType.Relu)
                nc.vector.tensor_reduce(
                    out=feats[e][:, s0:s0 + segs],
                    in_=zr[:, :w].rearrange("p (s j) -> p s j", j=J),
                    axis=mybir.AxisListType.X,
                    op=mybir.AluOpType.add,
                )
                if conv is not None:
                    # this e-tile's feats columns for the finished MM2
                    # round are final; convert to bf16 so MM2 never waits
                    lo, hi = conv
                    nc.vector.tensor_copy(featsb[e][:, lo:hi],
                                          feats[e][:, lo:hi])

            wc_sb = None

            with tc.tile_pool(name="ps1", bufs=8, space="PSUM") as pp1:
                # warm-up matmuls: all into one scratch psum slot, no
                # consumers; they only keep the PE array busy.
                ps_warm = pp1.tile([64, 64], f32, tag="ps", name="ps_warm")
                for _ in range(WARMUP_MMS):
                    nc.tensor.matmul(ps_warm[:, :], warm[:, 0:64],
                                     warm[:, 64:128], start=True, stop=True)

                n_dt = 2 * KT if mode == "fp8" else KT
                for ci, (t0, w) in enumerate(chunks):
                    xt = px.tile([128, n_dt, CHUNK], in_dt, tag="xt", name="xt")
                    cv = xt_d[D * t0:D * (t0 + w)].rearrange(
                        "(p d t) -> p d t", p=128, d=n_dt)
                    if ci == 0:
                        # chunk 0 rides Scalar's HWDGE in k-pairs so the
                        # first slices land while Sync streams w1[0]/w1[1];
                        # j3 goes on sync (behind j0/j1) instead of scalar
                        # so it lands well before the kt=3 matmuls need it.
                        for j in range(n_dt // 2):
                            eng = nc.scalar if j == n_dt // 4 else nc.sync
                            eng.dma_start(out=xt[:, 2 * j:2 * j + 2, :w],
                                          in_=cv[:, 2 * j:2 * j + 2, :])
                    else:
                        nc.sync.dma_start(out=xt[:, :, :w], in_=cv)
                    s0 = t0 // J
                    conv = None
                    if ci == 7:
                        conv = (0, R1_SEGS)
                    elif ci == len(chunks) - 1:
                        conv = (R1_SEGS, S_LOC)
                    if ci == 0:
                        # k-outer across parallel psum groups: first MMs
                        # only need w1_sb[0] + the first x k-slices.
                        e0 = 0
                        for wave in (8, 8):
                            pss = [pp1.tile([128, CHUNK], f32, tag="ps",
                                            name=f"ps0_{e0}_{i}")
                                   for i in range(wave)]
                            for kt in range(KT):
                                for i in range(wave):
                                    nc.tensor.matmul(
                                        pss[i][:, :w],
                                        lhsT(kt, e0 + i),
                                        rhs(xt, kt, w),
                                        start=(kt == 0),
                                        stop=(kt == KT - 1),
                                        perf_mode=perf,
                                    )
                            for i in range(wave):
                                relu_pool(pss[i], w, e0 + i, s0)
                            e0 += wave
                        continue
                    for e in range(E_TILES):
                        ps = pp1.tile([128, CHUNK], f32, tag="ps", name="ps")
                        for kt in range(KT):
                            nc.tensor.matmul(
                                ps[:, :w],
                                lhsT(kt, e),
                                rhs(xt, kt, w),
                                start=(kt == 0),
                                stop=(kt == KT - 1),
                                perf_mode=perf,
                            )
                        relu_pool(ps, w, e, s0, conv=conv)
                    if ci == 1:
                        # MM2 weights: issued late so they don't compete
                        # with W1/x for startup bandwidth.
                        wc_sb = []
                        for e in range(E_TILES):
                            t = pw.tile([128, C], bf16, tag=f"wc_{e}",
                                        name=f"wc_{e}")
                            nc.gpsimd.dma_start(
                                out=t, in_=wct_d[e * 128:(e + 1) * 128, :])
                            wc_sb.append(t)
                    if ci == 7:
                        # MM2 round 1: segs 0..127 are final.  The PE does
                        # these 32 matmuls between chunk 7 and chunk 8 (it
                        # is the serial resource either way) and the 500KB
                        # of output DMA drains mid-kernel instead of
                        # piling up after the last matmul.
                        ob1 = pw.tile([R1_SEGS, C], f32, tag="ob1", name="ob1")
                        for c0 in (0, 500):
                            ps = pp1.tile([R1_SEGS, 500], f32, tag="ps",
                                          name="mm2a")
                            for e in range(E_TILES):
                                nc.tensor.matmul(
                                    ps[:, :],
                                    featsb[e][:, 0:R1_SEGS],
                                    wc_sb[e][:, c0:c0 + 500],
                                    start=(e == 0),
                                    stop=(e == E_TILES - 1),
                                )
                            cs = slice(c0, c0 + 500)
                            nc.vector.tensor_copy(ob1[:, cs], ps[:, :])
                            nc.gpsimd.dma_start(out=out_d[0:R1_SEGS, cs],
                                                in_=ob1[:, cs])

                # MM2 round 2: segs 128..249, four 250-column banks so the
                # copy+DMA of bank q overlaps the matmuls of bank q+1 and
                # the final drain is a single [122, 250] strip per engine.
                ob2 = pw.tile([R2_SEGS, C], f32, tag="ob2", name="ob2")
                out_engs = [nc.scalar, nc.gpsimd, nc.sync, nc.gpsimd]
                for qi in range(4):
                    c0 = qi * 250
                    ps = pp1.tile([R2_SEGS, 250], f32, tag="ps", name="mm2b")
                    for e in range(E_TILES):
                        nc.tensor.matmul(
                            ps[:, :],
                            featsb[e][:, R1_SEGS:S_LOC],
                            wc_sb[e][:, c0:c0 + 250],
                            start=(e == 0),
                            stop=(e == E_TILES - 1),
                        )
                    cs = slice(c0, c0 + 250)
                    nc.scalar.copy(ob2[:, cs], ps[:, :])
                    out_engs[qi].dma_start(out=out_d[R1_SEGS:S_LOC, cs],
                                           in_=ob2[:, cs])

    nc.compile()
    return nc


def _prep_inputs(x, W1, Wlin, mode=MODE):
    wc = (Wlin[:, :E] + Wlin[:, E:]) / np.float32(J)     # [C, E] f32
    if mode == "fp8":
        in_np = FP8
        W1 = W1 * np.float32(W1_SCALE)
        wc = wc / np.float32(W1_SCALE)
    else:
        in_np = BF16
    wct = np.ascontiguousarray(wc.T).astype(BF16)        # [E, C] bf16
    w1t = np.ascontiguousarray(W1.T).astype(in_np)       # [D, E]
    in_maps = []
    for c in range(N_CORES):
        xs = x[c * T_LOC:(c + 1) * T_LOC]                # [7500, 1024]
        pieces = []
        t0 = 0
        for w in CHUNK_WIDTHS:                           # [p][d_tile][t] chunks
            blk = xs[t0:t0 + w].T.reshape(8, 128, w).transpose(1, 0, 2)
            pieces.append(np.ascontiguousarray(blk).astype(in_np).ravel())
            t0 += w
        xt = np.concatenate(pieces)                      # [D*T_LOC] flat
        in_maps.append({"xt": xt, "w1t": w1t, "wct": wct})
    return in_maps


def _run(in_maps, mode=MODE, trace=False, **kw):
    from concourse.bass_utils import run_bass_kernel_spmd

    if mode not in _cache:
        _cache[mode] = _build(mode)
    res = run_bass_kernel_spmd(_cache[mode], in_maps,
                               core_ids=list(range(N_CORES)), trace=trace, **kw)
    logits = np.concatenate([r["out"] for r in res.results], axis=0)
    return logits, res


def kernel(x, y, W1, W2, Wlin):
    x = np.asarray(x, dtype=np.float32)
    W1 = np.asarray(W1, dtype=np.float32)
    Wlin = np.asarray(Wlin, dtype=np.float32)
    modes = (MODE, "bf16") if MODE != "bf16" else ("bf16",)
    for i, mode in enumerate(modes):
        try:
            logits, _ = _run(_prep_inputs(x, W1, Wlin, mode=mode), mode=mode)
            return logits
        except Exception:
            if i == len(modes) - 1:
                raise
    raise RuntimeError("unreachable")


# revision 11
# speedup vs baseline: 1.0090x; 1.0090x over previous
"""Trainium2 Bass kernel for nn_Classifier_69818988363910 (segment_reduce).

Reference computation (after dead-code elimination):
    local = relu(x @ W1^T)                        # [60000, 2048]
    feats = local.reshape(2000, 30, 2048).mean(1) # [2000, 2048]
    logits = concat(feats, feats) @ Wlin^T        # [2000, 1000]
           = feats @ (Wlin[:, :2048] + Wlin[:, 2048:])^T
y / W2 are computed but unused in the reference (original-code bug), so the
output depends only on x, W1, Wlin.

Sharding: data-parallel over the 8 NeuronCores along T (7500 rows = 250
segments per core); W1 / Wc replicated. No collectives; host gathers.

Device kernel per core (fp32 accumulation in PSUM throughout):
    MM1 on PE:   z[e, t] = sum_d W1T[d, e] * xT[d, t]
                 bf16 mode: 8 k-tiles of 128;  fp8 mode: 4 DoubleRow
                 super-k-tiles of 256 (2x PE rate)
    relu on ACT: psum -> sbuf
    pool on DVE: tensor_reduce over [128, segs, 30] view (sum; the 1/30
                 mean scale and the fp8 W1 pre-scale are folded into Wc
                 on the host)
    MM2 on PE:   logits[s, c] = sum_e featsT[e, s] * WcT[e, c]  (bf16)

Schedule notes (tuned against the ntff profile):
  * ~40 dummy N=64 matmuls on a zero tile run during the startup DMA
    window so the PE HAM clock-gate un-throttles (1.2 -> 2.4 GHz)
    before the first real matmul.
  * Chunks are 14 x 480 + 2 x 390 so no chunk is narrow enough to be
    LDWEIGHTS-bound (the old 300-wide tail chunk was).
  * MM2 is split into two rounds: segs 0..127 right after chunk 7
    (its output DMA drains mid-kernel) and segs 128..249 at the end in
    four 250-column PSUM banks whose copy+DMA pipeline across engines,
    so almost no output traffic is left after the last matmul.
  * Chunk 0 is issued k-outer across 8 parallel PSUM groups so the PE
    can start as soon as the first weight k-tile lands.
"""

import os

import numpy as np
import ml_dtypes

BF16 = ml_dtypes.bfloat16
FP8 = ml_dtypes.float8_e4m3

MODE = os.environ.get("BASS_KERNEL_MODE", "fp8")    # "bf16" | "fp8"
W1_SCALE = 32.0                                     # fp8 mode: keep W1 out of subnormals

N_CORES = 8
T, D, E, C, J = 60000, 1024, 2048, 1000, 30
T_LOC = T // N_CORES          # 7500 rows per core
S_LOC = T_LOC // J            # 250 segments per core
CHUNK = 480                   # max t-chunk (psum bank limit is 512 f32)
CHUNK_WIDTHS = [480] * 14 + [390, 390]   # 14x16 + 2x13 = 250 segments
E_TILES = E // 128            # 16
R1_SEGS = 128                 # MM2 round 1: segs 0..127 (after chunk 7)
R2_SEGS = S_LOC - R1_SEGS     # MM2 round 2: segs 128..249 (122 rows)
WARMUP_MMS = 84               # dummy matmuls to flip the HAM clock gate

_cache = {}


def _build(mode):
    from concourse import bacc, mybir
    from concourse.tile import TileContext

    f32 = mybir.dt.float32
    bf16 = mybir.dt.bfloat16
    fp8 = mybir.dt.float8e4
    in_dt = fp8 if mode == "fp8" else bf16
    KT = 4 if mode == "fp8" else 8          # accumulation steps per psum group
    perf = mybir.MatmulPerfMode.DoubleRow if mode == "fp8" else None

    nc = bacc.Bacc(trn_type="TRN2", target_bir_lowering=False, debug=False,
                   num_devices=N_CORES, num_swdge_queues=4)

    # x shard pre-chunked on the host to [chunk][p=128][d_tile][t] so each
    # chunk is one partition-contiguous DMA
    xt_d = nc.declare_dram_parameter("xt", [D * T_LOC], in_dt, isOutput=False)
    w1t_d = nc.declare_dram_parameter("w1t", [D, E], in_dt, isOutput=False)
    wct_d = nc.declare_dram_parameter("wct", [E, C], bf16, isOutput=False)
    out_d = nc.declare_dram_parameter("out", [S_LOC, C], f32, isOutput=True)

    chunks = []
    t0 = 0
    for w in CHUNK_WIDTHS:
        chunks.append((t0, w))
        t0 += w
    assert t0 == T_LOC

    if mode == "fp8":
        # weight super-k-tile kt covers d = kt*256 + j*128 + p
        w1t_v = w1t_d[:, :].rearrange("(kt j p) e -> p kt j e", j=2, p=128)
    else:
        w1t_v = w1t_d[:, :].rearrange("(kt p) e -> p kt e", p=128)

    with TileContext(nc) as tc:
        with (
            tc.tile_pool(name="xin", bufs=2) as px,
            tc.tile_pool(name="wgt", bufs=1) as pw,
            tc.tile_pool(name="zrl", bufs=4) as pz,
        ):
            # --- PE warm-up: the HAM clock gate keeps the PE at 1.2 GHz
            # until ~3.4us of sustained matmul activity.  Burn that window
            # on dummy matmuls over a zeroed tile while the first weight /
            # x DMAs are still in flight, so every real matmul runs at
            # 2.4 GHz.  (vector does the memset: gpsimd is busy issuing
            # the W1 DMAs and scalar/sync feed the x chunks.)
            warm = pw.tile([128, 128], in_dt, tag="warm", name="warm")
            nc.vector.memset(warm, 0)

            # --- W1 startup supply.  All three DMA queue families ramp
            # slowly for the first ~5us (measured: gpsimd ~60-100 KB/us,
            # scalar ~50, sync ~100-160, reaching full rate only ~6us
            # after preamble-end), so wave-1's weight k-tiles are spread
            # across ALL of them, ordered by the time the PE will need
            # each piece.  kt0 feeds the very first matmuls and is split
            # finely; kt1 rides sync (fastest early ramp) ahead of the x
            # chunks; kt2/kt3 ride gpsimd which has caught up by then.
            if mode == "fp8":
                w1_sb = [pw.tile([128, 2, E], fp8, tag=f"w1_{k}", name=f"w1_{k}")
                         for k in range(KT)]
            else:
                w1_sb = [pw.tile([128, E], bf16, tag=f"w1_{k}", name=f"w1_{k}")
                         for k in range(KT)]

            def w1_dma(eng, kt, hs):
                if mode == "fp8":
                    eng.dma_start(out=w1_sb[kt][:, :, hs], in_=w1t_v[:, kt, :, hs])
                else:
                    eng.dma_start(out=w1_sb[kt][:, hs], in_=w1t_v[:, kt, hs])

            h = E // 2
            w1_dma(nc.scalar, 0, slice(0, 256))        # first 2 e-tiles
            w1_dma(nc.scalar, 0, slice(256, 512))
            w1_dma(nc.gpsimd, 0, slice(512, h))
            sync_kts = [1] if KT == 4 else list(range(1, KT, 2))
            for k in range(2, KT) if KT == 4 else range(2, KT, 2):
                w1_dma(nc.gpsimd, k, slice(0, h))
            for k in range(KT):                        # wave 2 halves
                w1_dma(nc.gpsimd, k, slice(h, E))
            # kt1's wave-1 half rides sync, issued INSIDE the chunk-0
            # block so it queues behind x j0/j1 (which the PE needs
            # first) but ahead of the chunk-1..15 x streams.

            feats = [pw.tile([128, S_LOC], f32, tag=f"fs_{e}", name=f"fs_{e}")
                     for e in range(E_TILES)]

            def lhsT(kt, e):
                if mode == "fp8":
                    return w1_sb[kt][:, :, e * 128:(e + 1) * 128]
                return w1_sb[kt][:, e * 128:(e + 1) * 128]

            def rhs(xt, kt, w):
                if mode == "fp8":
                    return xt[:, 2 * kt:2 * kt + 2, :w]
                return xt[:, kt, :w]

            featsb = [pw.tile([128, S_LOC], bf16, tag=f"fb_{e}", name=f"fb_{e}")
                      for e in range(E_TILES)]

            def relu_pool(ps, w, e, s0, conv=None):
                segs = w // J
                zr = pz.tile([128, CHUNK], f32, tag="zr", name="zr")
                nc.scalar.activation(zr[:, :w], ps[:, :w],
                                     mybir.ActivationFunctionType.Relu)
                nc.vector.tensor_reduce(
                    out=feats[e][:, s0:s0 + segs],
                    in_=zr[:, :w].rearrange("p (s j) -> p s j", j=J),
                    axis=mybir.AxisListType.X,
                    op=mybir.AluOpType.add,
                )
                if conv is not None:
                    # this e-tile's feats columns for the finished MM2
                    # round are final; convert to bf16 so MM2 never waits
                    lo, hi = conv
                    nc.vector.tensor_copy(featsb[e][:, lo:hi],
                                          feats[e][:, lo:hi])

            wc_sb = None

            with tc.tile_pool(name="ps1", bufs=8, space="PSUM") as pp1:
                # warm-up matmuls: all into one scratch psum slot, no
                # consumers; they only keep the PE array busy.
                ps_warm = pp1.tile([64, 64], f32, tag="ps", name="ps_warm")
                for _ in range(WARMUP_MMS):
                    nc.tensor.matmul(ps_warm[:, :], warm[:, 0:64],
                                     warm[:, 64:128], start=True, stop=True)

                n_dt = 2 * KT if mode == "fp8" else KT
                for ci, (t0, w) in enumerate(chunks):
                    xt = px.tile([128, n_dt, CHUNK], in_dt, tag="xt", name="xt")
                    cv = xt_d[D * t0:D * (t0 + w)].rearrange(
                        "(p d t) -> p d t", p=128, d=n_dt)
                    if ci == 0:
                        # chunk 0 in k-pairs.  Sync's queue order is
                        # j0, j1, kt1-weights, j3 — each lands with
                        # >=1us of margin before the PE needs it; j2
                        # rides scalar behind the kt0 weight pieces.
                        def xpair(eng, j):
                            eng.dma_start(out=xt[:, 2 * j:2 * j + 2, :w],
                                          in_=cv[:, 2 * j:2 * j + 2, :])
                        xpair(nc.sync, 0)
                        xpair(nc.sync, 1)
                        xpair(nc.scalar, 2)
                        for k in sync_kts:
                            w1_dma(nc.sync, k, slice(0, E // 2))
                        xpair(nc.sync, 3)
                    else:
                        nc.sync.dma_start(out=xt[:, :, :w], in_=cv)
                    s0 = t0 // J
                    conv = None
                    if ci == 7:
                        conv = (0, R1_SEGS)
                    elif ci == len(chunks) - 1:
                        conv = (R1_SEGS, S_LOC)
                    if ci == 0:
                        # k-outer across parallel psum groups: first MMs
                        # only need w1_sb[0] + the first x k-slices.
                        e0 = 0
                        for wave in (8, 8):
                            pss = [pp1.tile([128, CHUNK], f32, tag="ps",
                                            name=f"ps0_{e0}_{i}")
                                   for i in range(wave)]
                            for kt in range(KT):
                                for i in range(wave):
                                    nc.tensor.matmul(
                                        pss[i][:, :w],
                                        lhsT(kt, e0 + i),
                                        rhs(xt, kt, w),
                                        start=(kt == 0),
                                        stop=(kt == KT - 1),
                                        perf_mode=perf,
                                    )
                            for i in range(wave):
                                relu_pool(pss[i], w, e0 + i, s0)
                            e0 += wave
                        continue
                    for e in range(E_TILES):
                        ps = pp1.tile([128, CHUNK], f32, tag="ps", name="ps")
                        for kt in range(KT):
                            nc.tensor.matmul(
                                ps[:, :w],
                                lhsT(kt, e),
                                rhs(xt, kt, w),
                                start=(kt == 0),
                                stop=(kt == KT - 1),
                                perf_mode=perf,
                            )
                        relu_pool(ps, w, e, s0, conv=conv)
                    if ci == 1:
                        # MM2 weights: issued late so they don't compete
                        # with W1/x for startup bandwidth.
                        wc_sb = []
                        for e in range(E_TILES):
                            t = pw.tile([128, C], bf16, tag=f"wc_{e}",
                                        name=f"wc_{e}")
                            nc.gpsimd.dma_start(
                                out=t, in_=wct_d[e * 128:(e + 1) * 128, :])
                            wc_sb.append(t)
                    if ci == 7:
                        # MM2 round 1: segs 0..127 are final.  The PE does
                        # these 32 matmuls between chunk 7 and chunk 8 (it
                        # is the serial resource either way) and the 500KB
                        # of output DMA drains mid-kernel instead of
                        # piling up after the last matmul.
                        ob1 = pw.tile([R1_SEGS, C], f32, tag="ob1", name="ob1")
                        for c0 in (0, 500):
                            ps = pp1.tile([R1_SEGS, 500], f32, tag="ps",
                                          name="mm2a")
                            for e in range(E_TILES):
                                nc.tensor.matmul(
                                    ps[:, :],
                                    featsb[e][:, 0:R1_SEGS],
                                    wc_sb[e][:, c0:c0 + 500],
                                    start=(e == 0),
                                    stop=(e == E_TILES - 1),
                                )
                            cs = slice(c0, c0 + 500)
                            nc.vector.tensor_copy(ob1[:, cs], ps[:, :])
                            nc.scalar.dma_start(out=out_d[0:R1_SEGS, cs],
                                                in_=ob1[:, cs])

                # MM2 round 2: segs 128..249 in two 500-column banks; the
                # copy+DMA of bank 0 overlaps bank 1's matmuls.  Outputs
                # stay on scalar's HWDGE: [*, 500] f32 rows are 2000B
                # packets which that queue family spreads widest (narrow
                # strips and the gpsimd/sync queues both drain several
                # times slower, measured).
                ob2 = pw.tile([R2_SEGS, C], f32, tag="ob2", name="ob2")
                for c0 in (0, 500):
                    ps = pp1.tile([R2_SEGS, 500], f32, tag="ps", name="mm2b")
                    for e in range(E_TILES):
                        nc.tensor.matmul(
                            ps[:, :],
                            featsb[e][:, R1_SEGS:S_LOC],
                            wc_sb[e][:, c0:c0 + 500],
                            start=(e == 0),
                            stop=(e == E_TILES - 1),
                        )
                    cs = slice(c0, c0 + 500)
                    nc.scalar.copy(ob2[:, cs], ps[:, :])
                    nc.scalar.dma_start(out=out_d[R1_SEGS:S_LOC, cs],
                                        in_=ob2[:, cs])

    nc.compile()
    return nc


def _prep_inputs(x, W1, Wlin, mode=MODE):
    wc = (Wlin[:, :E] + Wlin[:, E:]) / np.float32(J)     # [C, E] f32
    if mode == "fp8":
        in_np = FP8
        W1 = W1 * np.float32(W1_SCALE)
        wc = wc / np.float32(W1_SCALE)
    else:
        in_np = BF16
    wct = np.ascontiguousarray(wc.T).astype(BF16)        # [E, C] bf16
    w1t = np.ascontiguousarray(W1.T).astype(in_np)       # [D, E]
    in_maps = []
    for c in range(N_CORES):
        xs = x[c * T_LOC:(c + 1) * T_LOC]                # [7500, 1024]
        pieces = []
        t0 = 0
        for w in CHUNK_WIDTHS:                           # [p][d_tile][t] chunks
            blk = xs[t0:t0 + w].T.reshape(8, 128, w).transpose(1, 0, 2)
            pieces.append(np.ascontiguousarray(blk).astype(in_np).ravel())
            t0 += w
        xt = np.concatenate(pieces)                      # [D*T_LOC] flat
        in_maps.append({"xt": xt, "w1t": w1t, "wct": wct})
    return in_maps


def _run(in_maps, mode=MODE, trace=False, **kw):
    from concourse.bass_utils import run_bass_kernel_spmd

    if mode not in _cache:
        _cache[mode] = _build(mode)
    res = run_bass_kernel_spmd(_cache[mode], in_maps,
                               core_ids=list(range(N_CORES)), trace=trace, **kw)
    logits = np.concatenate([r["out"] for r in res.results], axis=0)
    return logits, res


def kernel(x, y, W1, W2, Wlin):
    x = np.asarray(x, dtype=np.float32)
    W1 = np.asarray(W1, dtype=np.float32)
    Wlin = np.asarray(Wlin, dtype=np.float32)
    modes = (MODE, "bf16") if MODE != "bf16" else ("bf16",)
    for i, mode in enumerate(modes):
        try:
            logits, _ = _run(_prep_inputs(x, W1, Wlin, mode=mode), mode=mode)
            return logits
        except Exception:
            if i == len(modes) - 1:
                raise
    raise RuntimeError("unreachable")


# revision 14
# speedup vs baseline: 1.0450x; 1.0357x over previous
"""Trainium2 Bass kernel for nn_Classifier_69818988363910 (segment_reduce).

Reference computation (after dead-code elimination):
    local = relu(x @ W1^T)                        # [60000, 2048]
    feats = local.reshape(2000, 30, 2048).mean(1) # [2000, 2048]
    logits = concat(feats, feats) @ Wlin^T        # [2000, 1000]
           = feats @ (Wlin[:, :2048] + Wlin[:, 2048:])^T
y / W2 are computed but unused in the reference (original-code bug), so the
output depends only on x, W1, Wlin.

Sharding: data-parallel over the 8 NeuronCores along T (7500 rows = 250
segments per core); W1 / Wc replicated. No collectives; host gathers.

Device kernel per core (fp32 accumulation in PSUM throughout):
    MM1 on PE:   z[e, t] = sum_d W1T[d, e] * xT[d, t]
                 bf16 mode: 8 k-tiles of 128;  fp8 mode: 4 DoubleRow
                 super-k-tiles of 256 (2x PE rate)
    relu on ACT: psum -> sbuf
    pool on DVE: tensor_reduce over [128, segs, 30] view (sum; the 1/30
                 mean scale and the fp8 W1 pre-scale are folded into Wc
                 on the host)
    MM2 on PE:   logits[s, c] = sum_e featsT[e, s] * WcT[e, c]  (bf16)

Schedule notes (tuned against the ntff profile):
  * ~40 dummy N=64 matmuls on a zero tile run during the startup DMA
    window so the PE HAM clock-gate un-throttles (1.2 -> 2.4 GHz)
    before the first real matmul.
  * Chunks are 14 x 480 + 2 x 390 so no chunk is narrow enough to be
    LDWEIGHTS-bound (the old 300-wide tail chunk was).
  * MM2 is split into two rounds: segs 0..127 right after chunk 7
    (its output DMA drains mid-kernel) and segs 128..249 at the end in
    four 250-column PSUM banks whose copy+DMA pipeline across engines,
    so almost no output traffic is left after the last matmul.
  * Chunk 0 is issued k-outer across 8 parallel PSUM groups so the PE
    can start as soon as the first weight k-tile lands.
"""

import os

import numpy as np
import ml_dtypes

BF16 = ml_dtypes.bfloat16
FP8 = ml_dtypes.float8_e4m3

MODE = os.environ.get("BASS_KERNEL_MODE", "fp8")    # "bf16" | "fp8"
W1_SCALE = 32.0                                     # fp8 mode: keep W1 out of subnormals

N_CORES = 8
T, D, E, C, J = 60000, 1024, 2048, 1000, 30
T_LOC = T // N_CORES          # 7500 rows per core
S_LOC = T_LOC // J            # 250 segments per core
CHUNK = 480                   # max t-chunk (psum bank limit is 512 f32)
CHUNK_WIDTHS = [480] * 14 + [390, 390]   # 14x16 + 2x13 = 250 segments
E_TILES = E // 128            # 16
R1_SEGS = 122                 # MM2 round 1: segs 0..121 (after chunk 7)
R2_SEGS = S_LOC - R1_SEGS     # MM2 round 2: segs 122..249 (128 rows)
# R2 is exactly 128 rows: full-128-partition DMA calls spread across all
# 16 hardware queues (measured), partial ones land on only 2, which is
# ~8x slower — and R2's drain is on the kernel's critical path.
WARMUP_MMS = 84               # dummy matmuls to flip the HAM clock gate

_cache = {}


def _build(mode):
    from concourse import bacc, mybir
    from concourse.tile import TileContext

    f32 = mybir.dt.float32
    bf16 = mybir.dt.bfloat16
    fp8 = mybir.dt.float8e4
    in_dt = fp8 if mode == "fp8" else bf16
    KT = 4 if mode == "fp8" else 8          # accumulation steps per psum group
    perf = mybir.MatmulPerfMode.DoubleRow if mode == "fp8" else None

    nc = bacc.Bacc(trn_type="TRN2", target_bir_lowering=False, debug=False,
                   num_devices=N_CORES, num_swdge_queues=4)

    # x shard pre-chunked on the host to [chunk][p=128][d_tile][t] so each
    # chunk is one partition-contiguous DMA
    xt_d = nc.declare_dram_parameter("xt", [D * T_LOC], in_dt, isOutput=False)
    w1t_d = nc.declare_dram_parameter("w1t", [D, E], in_dt, isOutput=False)
    wct_d = nc.declare_dram_parameter("wct", [E, C], bf16, isOutput=False)
    out_d = nc.declare_dram_parameter("out", [S_LOC, C], f32, isOutput=True)

    chunks = []
    t0 = 0
    for w in CHUNK_WIDTHS:
        chunks.append((t0, w))
        t0 += w
    assert t0 == T_LOC

    if mode == "fp8":
        # weight super-k-tile kt covers d = kt*256 + j*128 + p
        w1t_v = w1t_d[:, :].rearrange("(kt j p) e -> p kt j e", j=2, p=128)
    else:
        w1t_v = w1t_d[:, :].rearrange("(kt p) e -> p kt e", p=128)

    with TileContext(nc) as tc:
        with (
            tc.tile_pool(name="xin", bufs=2) as px,
            tc.tile_pool(name="wgt", bufs=1) as pw,
            tc.tile_pool(name="zrl", bufs=4) as pz,
        ):
            # --- PE warm-up: the HAM clock gate keeps the PE at 1.2 GHz
            # until ~3.4us of sustained matmul activity.  Burn that window
            # on dummy matmuls over a zeroed tile while the first weight /
            # x DMAs are still in flight, so every real matmul runs at
            # 2.4 GHz.  (vector does the memset: gpsimd is busy issuing
            # the W1 DMAs and scalar/sync feed the x chunks.)
            warm = pw.tile([128, 128], in_dt, tag="warm", name="warm")
            nc.vector.memset(warm, 0)

            # --- W1 startup supply.  All three DMA queue families ramp
            # slowly for the first ~5us (measured: gpsimd ~60-100 KB/us,
            # scalar ~50, sync ~100-160, reaching full rate only ~6us
            # after preamble-end), so wave-1's weight k-tiles are spread
            # across ALL of them, ordered by the time the PE will need
            # each piece.  kt0 feeds the very first matmuls and is split
            # finely; kt1 rides sync (fastest early ramp) ahead of the x
            # chunks; kt2/kt3 ride gpsimd which has caught up by then.
            if mode == "fp8":
                w1_sb = [pw.tile([128, 2, E], fp8, tag=f"w1_{k}", name=f"w1_{k}")
                         for k in range(KT)]
            else:
                w1_sb = [pw.tile([128, E], bf16, tag=f"w1_{k}", name=f"w1_{k}")
                         for k in range(KT)]

            def w1_dma(eng, kt, hs):
                if mode == "fp8":
                    eng.dma_start(out=w1_sb[kt][:, :, hs], in_=w1t_v[:, kt, :, hs])
                else:
                    eng.dma_start(out=w1_sb[kt][:, hs], in_=w1t_v[:, kt, hs])

            h = E // 2
            w1_dma(nc.scalar, 0, slice(0, 256))        # first 2 e-tiles
            w1_dma(nc.scalar, 0, slice(256, 512))
            w1_dma(nc.gpsimd, 0, slice(512, h))
            sync_kts = [1, 3] if KT == 4 else list(range(1, KT, 2))
            for k in [2] if KT == 4 else range(2, KT, 2):
                w1_dma(nc.gpsimd, k, slice(0, h))
            for k in range(KT):                        # wave 2 halves
                w1_dma(nc.gpsimd, k, slice(h, E))
            # kt1's wave-1 half rides sync, issued INSIDE the chunk-0
            # block so it queues behind x j0/j1 (which the PE needs
            # first) but ahead of the chunk-1..15 x streams.

            feats = [pw.tile([128, S_LOC], f32, tag=f"fs_{e}", name=f"fs_{e}")
                     for e in range(E_TILES)]

            def lhsT(kt, e):
                if mode == "fp8":
                    return w1_sb[kt][:, :, e * 128:(e + 1) * 128]
                return w1_sb[kt][:, e * 128:(e + 1) * 128]

            def rhs(xt, kt, w):
                if mode == "fp8":
                    return xt[:, 2 * kt:2 * kt + 2, :w]
                return xt[:, kt, :w]

            featsb = [pw.tile([128, S_LOC], bf16, tag=f"fb_{e}", name=f"fb_{e}")
                      for e in range(E_TILES)]

            def relu_pool(ps, w, e, s0, conv=None):
                segs = w // J
                zr = pz.tile([128, CHUNK], f32, tag="zr", name="zr")
                nc.scalar.activation(zr[:, :w], ps[:, :w],
                                     mybir.ActivationFunctionType.Relu)
                nc.vector.tensor_reduce(
                    out=feats[e][:, s0:s0 + segs],
                    in_=zr[:, :w].rearrange("p (s j) -> p s j", j=J),
                    axis=mybir.AxisListType.X,
                    op=mybir.AluOpType.add,
                )
                if conv is not None:
                    # this e-tile's feats columns for the finished MM2
                    # round are final; convert to bf16 so MM2 never waits
                    lo, hi = conv
                    nc.vector.tensor_copy(featsb[e][:, lo:hi],
                                          feats[e][:, lo:hi])

            wc_sb = None

            with tc.tile_pool(name="ps1", bufs=8, space="PSUM") as pp1:
                # warm-up matmuls: all into one scratch psum slot, no
                # consumers; they only keep the PE array busy.
                ps_warm = pp1.tile([64, 64], f32, tag="ps", name="ps_warm")
                for _ in range(WARMUP_MMS):
                    nc.tensor.matmul(ps_warm[:, :], warm[:, 0:64],
                                     warm[:, 64:128], start=True, stop=True)

                n_dt = 2 * KT if mode == "fp8" else KT
                for ci, (t0, w) in enumerate(chunks):
                    xt = px.tile([128, n_dt, CHUNK], in_dt, tag="xt", name="xt")
                    cv = xt_d[D * t0:D * (t0 + w)].rearrange(
                        "(p d t) -> p d t", p=128, d=n_dt)
                    if ci == 0:
                        # chunk 0 in k-pairs.  Sync's queue order is
                        # j0, j1, kt1-weights, j3 — each lands with
                        # >=1us of margin before the PE needs it; j2
                        # rides scalar behind the kt0 weight pieces.
                        def xpair(eng, j):
                            eng.dma_start(out=xt[:, 2 * j:2 * j + 2, :w],
                                          in_=cv[:, 2 * j:2 * j + 2, :])
                        xpair(nc.sync, 0)
                        xpair(nc.sync, 1)
                        xpair(nc.scalar, 2)
                        for k in sync_kts:
                            w1_dma(nc.sync, k, slice(0, E // 2))
                        xpair(nc.sync, 3)
                    else:
                        nc.sync.dma_start(out=xt[:, :, :w], in_=cv)
                    s0 = t0 // J
                    conv = None
                    if ci == 7:
                        conv = (0, 128)       # segs 0..127 final after ci 7
                    elif ci == len(chunks) - 1:
                        conv = (128, S_LOC)
                    if ci == 0:
                        # k-outer across parallel psum groups: first MMs
                        # only need w1_sb[0] + the first x k-slices.
                        e0 = 0
                        for wave in (8, 8):
                            pss = [pp1.tile([128, CHUNK], f32, tag="ps",
                                            name=f"ps0_{e0}_{i}")
                                   for i in range(wave)]
                            for kt in range(KT):
                                for i in range(wave):
                                    nc.tensor.matmul(
                                        pss[i][:, :w],
                                        lhsT(kt, e0 + i),
                                        rhs(xt, kt, w),
                                        start=(kt == 0),
                                        stop=(kt == KT - 1),
                                        perf_mode=perf,
                                    )
                            for i in range(wave):
                                relu_pool(pss[i], w, e0 + i, s0)
                            e0 += wave
                        continue
                    for e in range(E_TILES):
                        ps = pp1.tile([128, CHUNK], f32, tag="ps", name="ps")
                        for kt in range(KT):
                            nc.tensor.matmul(
                                ps[:, :w],
                                lhsT(kt, e),
                                rhs(xt, kt, w),
                                start=(kt == 0),
                                stop=(kt == KT - 1),
                                perf_mode=perf,
                            )
                        relu_pool(ps, w, e, s0, conv=conv)
                    if ci == 1:
                        # MM2 weights: issued late so they don't compete
                        # with W1/x for startup bandwidth.
                        wc_sb = []
                        for e in range(E_TILES):
                            t = pw.tile([128, C], bf16, tag=f"wc_{e}",
                                        name=f"wc_{e}")
                            nc.gpsimd.dma_start(
                                out=t, in_=wct_d[e * 128:(e + 1) * 128, :])
                            wc_sb.append(t)
                    if ci == 7:
                        # MM2 round 1: segs 0..127 are final.  The PE does
                        # these 32 matmuls between chunk 7 and chunk 8 (it
                        # is the serial resource either way) and the 500KB
                        # of output DMA drains mid-kernel instead of
                        # piling up after the last matmul.
                        ob1 = pw.tile([R1_SEGS, C], f32, tag="ob1", name="ob1")
                        for c0 in (0, 500):
                            ps = pp1.tile([R1_SEGS, 500], f32, tag="ps",
                                          name="mm2a")
                            for e in range(E_TILES):
                                nc.tensor.matmul(
                                    ps[:, :],
                                    featsb[e][:, 0:R1_SEGS],
                                    wc_sb[e][:, c0:c0 + 500],
                                    start=(e == 0),
                                    stop=(e == E_TILES - 1),
                                )
                            cs = slice(c0, c0 + 500)
                            nc.vector.tensor_copy(ob1[:, cs], ps[:, :])
                            nc.scalar.dma_start(out=out_d[0:R1_SEGS, cs],
                                                in_=ob1[:, cs])

                # MM2 round 2: segs 128..249 in two 500-column banks; the
                # copy+DMA of bank 0 overlaps bank 1's matmuls.  Outputs
                # stay on scalar's HWDGE: [*, 500] f32 rows are 2000B
                # packets which that queue family spreads widest (narrow
                # strips and the gpsimd/sync queues both drain several
                # times slower, measured).
                ob2 = pw.tile([R2_SEGS, C], f32, tag="ob2", name="ob2")
                for c0 in (0, 500):
                    ps = pp1.tile([R2_SEGS, 500], f32, tag="ps", name="mm2b")
                    for e in range(E_TILES):
                        nc.tensor.matmul(
                            ps[:, :],
                            featsb[e][:, R1_SEGS:S_LOC],
                            wc_sb[e][:, c0:c0 + 500],
                            start=(e == 0),
                            stop=(e == E_TILES - 1),
                        )
                    cs = slice(c0, c0 + 500)
                    nc.scalar.copy(ob2[:, cs], ps[:, :])
                    nc.scalar.dma_start(out=out_d[R1_SEGS:S_LOC, cs],
                                        in_=ob2[:, cs])

    nc.compile()
    return nc


def _prep_inputs(x, W1, Wlin, mode=MODE):
    wc = (Wlin[:, :E] + Wlin[:, E:]) / np.float32(J)     # [C, E] f32
    if mode == "fp8":
        in_np = FP8
        W1 = W1 * np.float32(W1_SCALE)
        wc = wc / np.float32(W1_SCALE)
    else:
        in_np = BF16
    wct = np.ascontiguousarray(wc.T).astype(BF16)        # [E, C] bf16
    w1t = np.ascontiguousarray(W1.T).astype(in_np)       # [D, E]
    in_maps = []
    for c in range(N_CORES):
        xs = x[c * T_LOC:(c + 1) * T_LOC]                # [7500, 1024]
        pieces = []
        t0 = 0
        for w in CHUNK_WIDTHS:                           # [p][d_tile][t] chunks
            blk = xs[t0:t0 + w].T.reshape(8, 128, w).transpose(1, 0, 2)
            pieces.append(np.ascontiguousarray(blk).astype(in_np).ravel())
            t0 += w
        xt = np.concatenate(pieces)                      # [D*T_LOC] flat
        in_maps.append({"xt": xt, "w1t": w1t, "wct": wct})
    return in_maps


def _run(in_maps, mode=MODE, trace=False, **kw):
    from concourse.bass_utils import run_bass_kernel_spmd

    if mode not in _cache:
        _cache[mode] = _build(mode)
    res = run_bass_kernel_spmd(_cache[mode], in_maps,
                               core_ids=list(range(N_CORES)), trace=trace, **kw)
    logits = np.concatenate([r["out"] for r in res.results], axis=0)
    return logits, res


def kernel(x, y, W1, W2, Wlin):
    x = np.asarray(x, dtype=np.float32)
    W1 = np.asarray(W1, dtype=np.float32)
    Wlin = np.asarray(Wlin, dtype=np.float32)
    modes = (MODE, "bf16") if MODE != "bf16" else ("bf16",)
    for i, mode in enumerate(modes):
        try:
            logits, _ = _run(_prep_inputs(x, W1, Wlin, mode=mode), mode=mode)
            return logits
        except Exception:
            if i == len(modes) - 1:
                raise
    raise RuntimeError("unreachable")
